# revision 36
# baseline (speedup 1.0000x reference)
"""SMPL (shape blend + pose blend + LBS skinning) Bass kernel for 8 TRN2 NeuronCores.

Data-parallel over batch: B=1024 -> 128 per core. All SMPL buffers replicated.

Main-loop engine split (all walls balanced ~equal):
  TensorE: vp via ONE fp8e4m3 DoubleRow matmul per c-plane (the whole K=232
           expanded coefficient space - template/shapedirs as fp8 hi+lo
           residual row pairs, posedirs x16 hi-only - contracted in a single
           512-col pass) + 12 bf16 T-plane matmuls (K=24).
  ScalarE: PSUM->SBUF bf16 copies (vp + 5 T pairs + plane 10).
  VectorE: plane-11 copy + 9 muls + 3 adds, double-FD over chunk PAIRS so
           bf16 2x_1P instructions amortize the 58-cycle DVE bubbles.
Software pipeline: chunk i's vp matmuls are emitted one period ahead of
chunk i-1's T/combine work so the in-order TensorE queue never head-blocks.

Preamble: pose/beta DMA first; Rodrigues with fused product tiles; J from a
host-precomputed [11,72] regressor (J = [1|beta] @ JS2); FK with k-major
scratch + tree adds; rest-pose correction fused k-major; paired G transposes.

Note: this machine pins the PE array at ~50% utilization (1.2 GHz effective,
verified: 192 continuous filler matmuls never warmed it), so matmul cost is
~N columns at 1.2 GHz regardless of dtype - DoubleRow wins by contracting
232 K-rows in one N-pass instead of two.

Output per core: [128, 3, 6890] bf16 plane-major; host reassembles [1024, 6890, 3].
"""

import sys
import numpy as np
import ml_dtypes

for _p in ("/opt/trn_rl_repo",):
    if _p not in sys.path:
        sys.path.append(_p)

import concourse.bass as bass
import concourse.tile as tile
import concourse.mybir as mybir
from concourse import bacc
from concourse.bass_utils import run_bass_kernel_spmd
from concourse.alu_op_type import AluOpType

F32 = mybir.dt.float32
BF16 = mybir.dt.bfloat16
F8E4 = mybir.dt.float8e4
NP_BF16 = ml_dtypes.bfloat16
NP_F8E4 = ml_dtypes.float8_e4m3
PD_SCALE = 16.0   # posedirs stored x16 in fp8; lrotmin coeffs stored /16

N_CORES = 8
B = 1024
B_LOC = B // N_CORES  # 128
NV = 6890
NJ = 24
NPD = 207         # pose blend coeffs
KC = 232          # expanded fp8 coeff: [1|beta]hi(11) [1|beta]lo(11) lrotmin(207) pad(3)
CH = 512          # vertex chunk

# FK level groups: (child_start, n_children, parent_start, parent_broadcast)
FK_GROUPS = [
    (1, 3, 0, True),
    (4, 3, 1, False),
    (7, 3, 4, False),
    (10, 3, 7, False),
    (13, 2, 9, True),
    (15, 3, 12, False),
    (18, 2, 16, False),
    (20, 2, 18, False),
    (22, 2, 20, False),
]

CFG = {
    "trace": False,
    "debug": False,
}

_CACHE = {}


def build_program(cfg):
    key = ("bf16", CH)
    if key in _CACHE:
        return _CACHE[key]

    nc = bacc.Bacc("TRN2", target_bir_lowering=False, debug=False)

    # ---- DRAM parameters ----
    pose_d = nc.dram_tensor("pose", [B_LOC, 72], F32, kind="ExternalInput")
    beta_d = nc.dram_tensor("beta", [B_LOC, 10], F32, kind="ExternalInput")
    dirs8_d = nc.dram_tensor("dirs8", [116, 2, 3, NV], F8E4, kind="ExternalInput")
    scal_d = nc.dram_tensor("scal", [116, 2], F32, kind="ExternalInput")
    wt_d = nc.dram_tensor("wt", [NJ, NV], BF16, kind="ExternalInput")
    js2_d = nc.dram_tensor("js2", [11, 72], F32, kind="ExternalInput")
    ident_d = nc.dram_tensor("ident", [128, 128], F32, kind="ExternalInput")
    out_d = nc.dram_tensor("out", [B_LOC, 3, NV], BF16, kind="ExternalOutput")

    with tile.TileContext(nc) as tc:
        with (
            tc.tile_pool(name="const", bufs=1) as constp,
            tc.tile_pool(name="state", bufs=1) as statep,
            tc.tile_pool(name="scr", bufs=1) as scrp,
        ):
            # ---- const loads (pose/beta first: they gate the serial preamble) ----
            pose_sb = statep.tile([B_LOC, 72], F32)
            nc.sync.dma_start(pose_sb[:, :], pose_d.ap())
            coeff = statep.tile([B_LOC, KC], F32)
            nc.sync.dma_start(coeff[:, 1:11], beta_d.ap())
            nc.sync.dma_start(coeff[:, 12:22], beta_d.ap())
            ident = constp.tile([128, 128], F32)
            nc.sync.dma_start(ident[:, :], ident_d.ap())
            js2_sb = constp.tile([11, 72], F32)
            nc.sync.dma_start(js2_sb[:, :], js2_d.ap())
            scal_sb = constp.tile([116, 2], F32)
            nc.sync.dma_start(scal_sb[:, :], scal_d.ap())
            wt_sb = constp.tile([NJ, NV], BF16)
            nc.sync.dma_start(wt_sb[:, :], wt_d.ap())

            # ---- Rodrigues (fp32) ----
            V = nc.vector
            S = nc.scalar
            sq = scrp.tile([B_LOC, 72], F32, tag="sq")
            V.tensor_mul(sq[:, :], pose_sb[:, :], pose_sb[:, :])
            sq3 = sq[:, :].rearrange("p (j c) -> p c j", c=3)
            th2 = scrp.tile([B_LOC, NJ], F32, tag="th2")
            V.tensor_add(th2[:, :], sq3[:, 0, :], sq3[:, 1, :])
            V.tensor_add(th2[:, :], th2[:, :], sq3[:, 2, :])
            cbias = constp.tile([128, 2], F32)
            V.memset(cbias[:, 0:1], 1e-8)
            V.memset(cbias[:, 1:2], float(np.pi / 2))
            theta = scrp.tile([B_LOC, NJ], F32, tag="theta")
            S.activation(theta[:, :], th2[:, :], mybir.ActivationFunctionType.Sqrt,
                         bias=cbias[0:B_LOC, 0:1])
            invt = scrp.tile([B_LOC, NJ], F32, tag="invt")
            V.reciprocal(invt[:, :], theta[:, :])
            sh = scrp.tile([B_LOC, NJ], F32, tag="sh")
            S.activation(sh[:, :], theta[:, :], mybir.ActivationFunctionType.Sin, scale=0.5)
            chh = scrp.tile([B_LOC, NJ], F32, tag="chh")
            S.activation(chh[:, :], theta[:, :], mybir.ActivationFunctionType.Sin,
                         scale=0.5, bias=cbias[0:B_LOC, 1:2])
            s_t = scrp.tile([B_LOC, NJ], F32, tag="s_t")
            V.scalar_tensor_tensor(s_t[:, :], sh[:, :], 2.0, chh[:, :], AluOpType.mult, AluOpType.mult)
            shsq = scrp.tile([B_LOC, NJ], F32, tag="shsq")
            V.tensor_mul(shsq[:, :], sh[:, :], sh[:, :])
            c_t = scrp.tile([B_LOC, NJ], F32, tag="c_t")
            V.tensor_scalar(c_t[:, :], shsq[:, :], -2.0, 1.0, AluOpType.mult, AluOpType.add)
            omc = scrp.tile([B_LOC, NJ], F32, tag="omc")
            V.tensor_scalar_mul(omc[:, :], shsq[:, :], 2.0)
            ax = scrp.tile([B_LOC, 72], F32, tag="ax")
            ax3 = ax[:, :].rearrange("p (j c) -> p c j", c=3)
            p3 = pose_sb[:, :].rearrange("p (j c) -> p c j", c=3)
            V.tensor_mul(ax3[:, :, :], p3[:, :, :],
                         invt[:, :].unsqueeze(1).broadcast_to([B_LOC, 3, NJ]))
            # products in one tile: [xx yy zz xy xz yz sx sy sz]
            t9 = scrp.tile([B_LOC, 9, NJ], F32, tag="t9")
            V.tensor_mul(t9[:, 0:3, :], ax3[:, :, :], ax3[:, :, :])
            V.tensor_mul(t9[:, 3:5, :],
                         ax3[:, 0:1, :].broadcast_to([B_LOC, 2, NJ]), ax3[:, 1:3, :])
            V.tensor_mul(t9[:, 5:6, :], ax3[:, 1:2, :], ax3[:, 2:3, :])
            V.tensor_mul(t9[:, 0:6, :], t9[:, 0:6, :],
                         omc[:, :].unsqueeze(1).broadcast_to([B_LOC, 6, NJ]))
            V.tensor_mul(t9[:, 6:9, :],
                         s_t[:, :].unsqueeze(1).broadcast_to([B_LOC, 3, NJ]),
                         ax3[:, :, :])
            prods = {n: t9[:, i, :] for i, n in enumerate(
                ["xx", "yy", "zz", "xy", "xz", "yz", "sx", "sy", "sz"])}
            r9 = statep.tile([B_LOC, NJ * 9], F32)
            r9e = r9[:, :].rearrange("p (j e) -> p e j", e=9)
            ENTRIES = [
                ("add", "c", "xx"), ("sub", "xy", "sz"), ("add", "xz", "sy"),
                ("add", "xy", "sz"), ("add", "c", "yy"), ("sub", "yz", "sx"),
                ("sub", "xz", "sy"), ("add", "yz", "sx"), ("add", "c", "zz"),
            ]
            for e, (op, a, b_) in enumerate(ENTRIES):
                ta = c_t[:, :] if a == "c" else prods[a]
                fn = V.tensor_add if op == "add" else V.tensor_sub
                fn(r9e[:, e, :], ta, prods[b_])

            # ---- coeff: [1|beta]hi [1|beta]lo lrotmin pad ----
            V.memset(coeff[:, 0:1], 1.0)
            V.memset(coeff[:, 11:12], 1.0)
            V.memset(coeff[:, 229:232], 0.0)
            V.tensor_copy(coeff[:, 22:229], r9[:, 9:216])
            lr9 = coeff[:, 22:229].rearrange("p (j e) -> p e j", e=9)
            for e in (0, 4, 8):
                V.tensor_scalar_add(lr9[:, e, :], lr9[:, e, :], -1.0)

            with tc.tile_pool(name="psA", bufs=2, space="PSUM") as psA:
                # full-coeff fp8 DoubleRow lhsT: expanded rows 0..115 from the
                # first transpose window, 116..231 from the second; per-row
                # power-of-2 scales delivered via scal_sb
                ptA = psA.tile([128, 128], F32, tag="tp")
                nc.tensor.transpose(ptA[:, :], coeff[:, 0:128], ident[:, :])
                betaT1 = statep.tile([11, B_LOC], F32)
                V.tensor_copy(betaT1[:, :], ptA[0:11, :])
                coeffT_b8 = statep.tile([116, 2, B_LOC], F8E4)
                V.tensor_scalar_mul(coeffT_b8[:, 0, :], ptA[0:116, :],
                                    scal_sb[:, 0:1])
                ptB = psA.tile([128, 128], F32, tag="tp")
                nc.tensor.transpose(ptB[0:116, :], coeff[:, 116:232], ident[:, :])
                V.tensor_scalar_mul(coeffT_b8[:, 1, :], ptB[0:116, :],
                                    scal_sb[:, 1:2])

                # ---- J = [1 | beta] @ JS2 (host-precomputed regressor) ----
                pj = psA.tile([B_LOC, 72], F32, tag="pj")
                nc.tensor.matmul(pj[:, :], betaT1[:, :], js2_sb[:, :],
                                 start=True, stop=True)
                j_sb = statep.tile([B_LOC, 72], F32)
                V.tensor_copy(j_sb[:, :], pj[:, :])

            # ---- J_rel ----
            jrel = statep.tile([B_LOC, 72], F32)
            jv = j_sb[:, :].rearrange("p (j c) -> p j c", c=3)
            jrv = jrel[:, :].rearrange("p (j c) -> p j c", c=3)
            V.tensor_copy(jrel[:, 0:3], j_sb[:, 0:3])
            V.tensor_sub(jrv[:, 1:4], jv[:, 1:4], jv[:, 0:1].broadcast_to([B_LOC, 3, 3]))
            V.tensor_sub(jrv[:, 4:12], jv[:, 4:12], jv[:, 1:9])
            V.tensor_sub(jrv[:, 12:15], jv[:, 12:15], jv[:, 9:10].broadcast_to([B_LOC, 3, 3]))
            V.tensor_sub(jrv[:, 15:18], jv[:, 15:18], jv[:, 12:15])
            V.tensor_sub(jrv[:, 18:24], jv[:, 18:24], jv[:, 16:22])

            # ---- local transforms Gl [128, 24*12] (3x4 row-major [R|t]) ----
            gl = statep.tile([B_LOC, NJ * 12], F32)
            gl4 = gl[:, :].rearrange("p (j m n) -> p j m n", m=3, n=4)
            r94 = r9[:, :].rearrange("p (j m n) -> p j m n", m=3, n=3)
            V.tensor_copy(gl4[:, :, :, 0:3], r94[:, :, :, :])
            V.tensor_copy(gl4[:, :, :, 3:4], jrv[:, :, :].unsqueeze(3))

            # ---- forward kinematics ----
            gw = statep.tile([B_LOC, NJ * 12], F32)
            gw4 = gw[:, :].rearrange("p (j m n) -> p j m n", m=3, n=4)
            V.tensor_copy(gw[:, 0:12], gl[:, 0:12])
            fk3 = scrp.tile([B_LOC, 3 * 3 * 12], F32, tag="fk3")
            for (c0, ncld, p0, bc) in FK_GROUPS:
                child = gw4[:, c0:c0 + ncld]
                loc = gl4[:, c0:c0 + ncld]
                par = gw4[:, p0:p0 + (1 if bc else ncld)]
                shp = [B_LOC, ncld, 3, 4]
                # per-k muls into a k-major scratch (APs are limited to 3
                # free dims), then tree adds
                f3 = fk3[:, 0:3 * ncld * 12].rearrange(
                    "p (k j m n) -> p k j m n", k=3, m=3, n=4)
                for k in range(3):
                    in0 = loc[:, :, k:k + 1, :].broadcast_to(shp)
                    pk = par[:, 0:1, :, k:k + 1] if bc else par[:, :, :, k:k + 1]
                    V.tensor_mul(f3[:, k], in0, pk.broadcast_to(shp))
                V.tensor_add(child[:, :, :, :], f3[:, 0], f3[:, 1])
                V.tensor_add(child[:, :, :, :], child[:, :, :, :], f3[:, 2])
                ptr = par[:, 0:1, :, 3:4] if bc else par[:, :, :, 3:4]
                V.tensor_add(child[:, :, :, 3:4], child[:, :, :, 3:4],
                             ptr.broadcast_to([B_LOC, ncld, 3, 1]))

            # ---- rest-pose correction: t_j -= R_j^w @ J_j (k-major fused) ----
            ct3 = scrp.tile([B_LOC, 3 * NJ * 3], F32, tag="ct3")
            c3v = ct3[:, :].rearrange("p (k j m) -> p k j m", k=3, m=3)
            cshp = [B_LOC, 3, NJ, 3]
            V.tensor_mul(c3v,
                         gw4[:, :, :, 0:3].rearrange("p j m k -> p k j m"),
                         jv[:, :, :].rearrange("p j k -> p k j").unsqueeze(3)
                         .broadcast_to(cshp))
            V.tensor_add(c3v[:, 0], c3v[:, 0], c3v[:, 1])
            V.tensor_add(c3v[:, 0], c3v[:, 0], c3v[:, 2])
            V.tensor_sub(gw4[:, :, :, 3:4], gw4[:, :, :, 3:4],
                         c3v[:, 0].unsqueeze(3))

            # ---- gat via 12 transposes: [24, 12*128] bf16 ----
            gat = statep.tile([NJ, 12 * B_LOC], BF16)
            gwe = gw[:, :].rearrange("p (j e) -> p e j", e=12)
            with tc.tile_pool(name="psT", bufs=3, space="PSUM") as psT:
                for g in range(6):
                    pgt = psT.tile([NJ, 2, B_LOC], F32, tag="gt")
                    for h in range(2):
                        nc.tensor.transpose(pgt[:, h, :], gwe[:, 2 * g + h, :],
                                            ident[:, :])
                    V.tensor_copy(
                        gat[:, 2 * g * B_LOC:(2 * g + 2) * B_LOC]
                        .rearrange("p (h v) -> p h v", v=B_LOC),
                        pgt[:, :, :])

            # ---- main vertex-chunk loop ----
            _main_loop(nc, tc, coeffT_b8, gat, wt_sb, dirs8_d, out_d)

    nc.compile()
    _CACHE[key] = nc
    return nc


def _main_loop(nc, tc, coeffT_b8, gat, wt_sb, dirs8_d, out_d):
    V = nc.vector
    S = nc.scalar
    ch = CH
    n_chunks = (NV + ch - 1) // ch

    with (
        tc.tile_pool(name="loop", bufs=4) as loopp,
        tc.tile_pool(name="pair", bufs=2) as pairp,
        tc.tile_pool(name="psVP", bufs=1, space="PSUM") as psVP,   # 3 banks
        tc.tile_pool(name="psTT", bufs=2, space="PSUM") as psTT,   # 2x2 banks
    ):
        state = {}

        def front_half(ci):
            """dirs DMA + vp matmuls + vp copy (ScalarE) for chunk ci.

            vp lands in slot ci%2 of a pair-wide tile so the combine can run
            double-FD instructions over two adjacent chunks."""
            v0 = ci * ch
            sz = min(ch, NV - v0)
            k = ci % 2
            if k == 0:
                state[("vp", ci // 2)] = pairp.tile([B_LOC, 3, 2, ch], BF16,
                                                    tag="vp", name="vp_pair")
            vp_pair = state[("vp", ci // 2)]
            db8 = loopp.tile([116, 2, 3, ch], F8E4, tag="db8")
            nc.sync.dma_start(db8[:, :, :, 0:sz], dirs8_d.ap()[:, :, :, v0:v0 + sz])
            pvp = psVP.tile([128, 3 * ch], F32, tag="pvp")
            for c in range(3):
                nc.tensor.matmul(pvp[:, c * ch:c * ch + sz], coeffT_b8[:, :, :],
                                 db8[:, :, c, 0:sz], start=True, stop=True,
                                 perf_mode=mybir.MatmulPerfMode.DoubleRow)
            if sz == ch:
                S.copy(vp_pair[:, :, k, :],
                       pvp[:, :].rearrange("p (c v) -> p c v", v=ch))
            else:
                for c in range(3):
                    S.copy(vp_pair[:, c, k, 0:sz], pvp[:, c * ch:c * ch + sz])

        def t_half(ci):
            """T matmuls + PSUM->SBUF copies for chunk ci into slot ci%2."""
            v0 = ci * ch
            sz = min(ch, NV - v0)
            k = ci % 2
            if k == 0:
                state[("t", ci // 2)] = pairp.tile([B_LOC, 12, 2, ch], BF16,
                                                   tag="t", name="t_pair")
            t_pair = state[("t", ci // 2)]
            for g in range(6):
                ptt = psTT.tile([B_LOC, 2 * ch], F32, tag="ptt")
                for h in range(2):
                    e = g * 2 + h
                    nc.tensor.matmul(ptt[:, h * ch:h * ch + sz],
                                     gat[:, e * B_LOC:(e + 1) * B_LOC],
                                     wt_sb[:, v0:v0 + sz], start=True, stop=True)
                if g < 5:
                    if sz == ch:
                        S.copy(t_pair[:, g * 2:g * 2 + 2, k, :],
                               ptt[:, :].rearrange("p (h v) -> p h v", v=ch))
                    else:
                        for h in range(2):
                            S.copy(t_pair[:, g * 2 + h, k, 0:sz],
                                   ptt[:, h * ch:h * ch + sz])
                else:
                    S.copy(t_pair[:, 10, k, 0:sz], ptt[:, 0:sz])
                    V.tensor_copy(t_pair[:, 11, k, 0:sz], ptt[:, ch:ch + sz])

        def combine(ci0, n_in_pair, szs):
            """double-FD combine over the pair (ci0, ci0+1); n_in_pair=1 for
            a lone tail chunk. szs: sizes of the pair's chunks."""
            v0 = ci0 * ch
            vp_pair = state.pop(("vp", ci0 // 2))
            t_pair = state.pop(("t", ci0 // 2))
            full = (n_in_pair == 2 and szs[1] == ch)
            w = 2 * ch if full else szs[0]  # combined free width when full
            pmul = pairp.tile([B_LOC, 3, 3, 2, ch], BF16, tag="pmul")
            q = pairp.tile([B_LOC, 3, 2, ch], BF16, tag="q")
            out_sb = pairp.tile([B_LOC, 3, 2, ch], BF16, tag="outsb")
            t3 = t_pair[:, :, :, :].rearrange("p (m n) k v -> p m n k v", n=4)[:, :, 3]

            if full:
                for c in range(3):
                    for m in range(3):
                        V.tensor_mul(pmul[:, c, m, :, :], t_pair[:, m * 4 + c, :, :],
                                     vp_pair[:, c, :, :])
                V.tensor_add(q[:, :, :, :], pmul[:, 0], pmul[:, 1])
                V.tensor_add(q[:, :, :, :], q[:, :, :, :], pmul[:, 2])
                V.tensor_add(out_sb[:, :, :, :], q[:, :, :, :], t3)
                nc.sync.dma_start(
                    out_d.ap()[:, :, v0:v0 + 2 * ch],
                    out_sb[:, :, :, :].rearrange("p c k v -> p c (k v)"))
            else:
                for kk in range(n_in_pair):
                    sz = szs[kk]
                    for c in range(3):
                        for m in range(3):
                            V.tensor_mul(pmul[:, c, m, kk, 0:sz],
                                         t_pair[:, m * 4 + c, kk, 0:sz],
                                         vp_pair[:, c, kk, 0:sz])
                    V.tensor_add(q[:, :, kk, 0:sz], pmul[:, 0, :, kk, 0:sz],
                                 pmul[:, 1, :, kk, 0:sz])
                    V.tensor_add(q[:, :, kk, 0:sz], q[:, :, kk, 0:sz],
                                 pmul[:, 2, :, kk, 0:sz])
                    V.tensor_add(out_sb[:, :, kk, 0:sz], q[:, :, kk, 0:sz],
                                 t3[:, :, kk, 0:sz])
                    nc.sync.dma_start(out_d.ap()[:, :, v0 + kk * ch:v0 + kk * ch + sz],
                                      out_sb[:, :, kk, 0:sz])

        # software pipeline, one chunk of lookahead on the vp side; combine
        # fires once per pair with double-FD DVE instructions.
        front_half(0)
        for ci in range(1, n_chunks):
            front_half(ci)
            t_half(ci - 1)
            if ci % 2 == 0:
                combine(ci - 2, 2, [ch, ch])
        t_half(n_chunks - 1)
        if n_chunks % 2 == 0:
            last0 = n_chunks - 2
            szs = [ch, min(ch, NV - (n_chunks - 1) * ch)]
            combine(last0, 2, szs)
        else:
            combine(n_chunks - 1, 1, [min(ch, NV - (n_chunks - 1) * ch), 0])


def _host_prep(inputs):
    shapedirs = np.asarray(inputs["shapedirs"], np.float32)    # [V,3,10]
    posedirs = np.asarray(inputs["posedirs"], np.float32)      # [V,3,207]
    v_template = np.asarray(inputs["v_template"], np.float32)  # [V,3]
    Jreg = np.asarray(inputs["J_regressor"], np.float32)       # [24,V]
    weights = np.asarray(inputs["weights"], np.float32)        # [V,24]

    def f8(x):
        return np.asarray(x, NP_F8E4)

    def dec(x):
        return f8(x).astype(np.float32)

    tmplr = v_template.T                      # [3, NV]
    sd = shapedirs.transpose(2, 1, 0)         # [10, 3, NV]
    pd = posedirs.transpose(2, 1, 0)          # [207, 3, NV]
    rows = np.zeros((232, 3, NV), np.float32)
    rows[0] = tmplr                           # hi rows store value at its scale
    rows[1:11] = 16 * sd
    rows[11] = 32 * (tmplr - dec(tmplr))      # residual rows
    rows[12:22] = 16 * (16 * sd - dec(16 * sd))
    rows[22:229] = 16 * pd
    dirs8 = f8(rows).reshape(2, 116, 3, NV).transpose(1, 0, 2, 3)
    scal = np.zeros(232, np.float32)
    scal[0] = 1.0
    scal[1:11] = 1.0 / 16
    scal[11] = 1.0 / 32
    scal[12:22] = 1.0 / 256
    scal[22:229] = 1.0 / 16
    scal2 = np.ascontiguousarray(scal.reshape(2, 116).T)
    # JS2: row 0 = Jreg @ template; rows 1..10 = Jreg @ shapedirs[:,:,s]
    js2 = np.empty((11, 72), np.float32)
    js2[0] = (Jreg @ v_template).reshape(72)
    js2[1:11] = np.einsum('jv,vcs->sjc', Jreg, shapedirs).reshape(10, 72)
    rep = {
        "dirs8": np.ascontiguousarray(dirs8),
        "scal": scal2,
        "wt": np.ascontiguousarray(weights.T.astype(NP_BF16)),
        "js2": js2,
        "ident": np.eye(128, dtype=np.float32),
    }
    return rep


def kernel(pose, beta, shapedirs, posedirs, v_template, J_regressor, weights):
    cfg = CFG
    nc = build_program(cfg)
    rep = _host_prep(dict(shapedirs=shapedirs, posedirs=posedirs, v_template=v_template,
                          J_regressor=J_regressor, weights=weights))
    pose = np.asarray(pose, np.float32)
    beta = np.asarray(beta, np.float32)
    in_maps = []
    for i in range(N_CORES):
        m = dict(rep)
        m["pose"] = np.ascontiguousarray(pose[i * B_LOC:(i + 1) * B_LOC])
        m["beta"] = np.ascontiguousarray(beta[i * B_LOC:(i + 1) * B_LOC])
        in_maps.append(m)
    res = run_bass_kernel_spmd(nc, in_maps, core_ids=list(range(N_CORES)),
                               trace=cfg.get("trace", False))
    kernel.last_results = res
    out = np.concatenate([np.asarray(res.results[i]["out"]).astype(np.float32)
                          for i in range(N_CORES)], axis=0)
    return np.ascontiguousarray(out.transpose(0, 2, 1))


# revision 38
# speedup vs baseline: 1.0167x; 1.0167x over previous
"""SMPL (shape blend + pose blend + LBS skinning) Bass kernel for 8 TRN2 NeuronCores.

Data-parallel over batch: B=1024 -> 128 per core. All SMPL buffers replicated.

Main-loop engine split (all walls balanced ~equal):
  TensorE: vp via ONE fp8e4m3 DoubleRow matmul per c-plane (the whole K=232
           expanded coefficient space - template/shapedirs as fp8 hi+lo
           residual row pairs, posedirs x16 hi-only - contracted in a single
           512-col pass) + 12 bf16 T-plane matmuls (K=24).
  ScalarE: PSUM->SBUF bf16 copies (vp + 5 T pairs + plane 10).
  VectorE: plane-11 copy + 9 muls + 3 adds, double-FD over chunk PAIRS so
           bf16 2x_1P instructions amortize the 58-cycle DVE bubbles.
Software pipeline: chunk i's vp matmuls are emitted one period ahead of
chunk i-1's T/combine work so the in-order TensorE queue never head-blocks.

Preamble: pose/beta DMA first; Rodrigues with fused product tiles; J from a
host-precomputed [11,72] regressor (J = [1|beta] @ JS2); FK with k-major
scratch + tree adds; rest-pose correction fused k-major; paired G transposes.

Note: this machine pins the PE array at ~50% utilization (1.2 GHz effective,
verified: 192 continuous filler matmuls never warmed it), so matmul cost is
~N columns at 1.2 GHz regardless of dtype - DoubleRow wins by contracting
232 K-rows in one N-pass instead of two.

Output per core: [128, 3, 6890] bf16 plane-major; host reassembles [1024, 6890, 3].
"""

import sys
import numpy as np
import ml_dtypes

for _p in ("/opt/trn_rl_repo",):
    if _p not in sys.path:
        sys.path.append(_p)

import concourse.bass as bass
import concourse.tile as tile
import concourse.mybir as mybir
from concourse import bacc
from concourse.bass_utils import run_bass_kernel_spmd
from concourse.alu_op_type import AluOpType

F32 = mybir.dt.float32
BF16 = mybir.dt.bfloat16
F8E4 = mybir.dt.float8e4
NP_BF16 = ml_dtypes.bfloat16
NP_F8E4 = ml_dtypes.float8_e4m3
PD_SCALE = 16.0   # posedirs stored x16 in fp8; lrotmin coeffs stored /16

N_CORES = 8
B = 1024
B_LOC = B // N_CORES  # 128
NV = 6890
NJ = 24
NPD = 207         # pose blend coeffs
KC = 232          # expanded fp8 coeff: [1|beta]hi(11) [1|beta]lo(11) lrotmin(207) pad(3)
CH = 512          # vertex chunk

# FK level groups: (child_start, n_children, parent_start, parent_broadcast)
FK_GROUPS = [
    (1, 3, 0, True),
    (4, 3, 1, False),
    (7, 3, 4, False),
    (10, 3, 7, False),
    (13, 2, 9, True),
    (15, 3, 12, False),
    (18, 2, 16, False),
    (20, 2, 18, False),
    (22, 2, 20, False),
]

CFG = {
    "trace": False,
    "debug": False,
}

_CACHE = {}


def build_program(cfg):
    key = ("bf16", CH)
    if key in _CACHE:
        return _CACHE[key]

    nc = bacc.Bacc("TRN2", target_bir_lowering=False, debug=False)

    # ---- DRAM parameters ----
    pose_d = nc.dram_tensor("pose", [B_LOC, 72], F32, kind="ExternalInput")
    beta_d = nc.dram_tensor("beta", [B_LOC, 10], F32, kind="ExternalInput")
    dirs8_d = nc.dram_tensor("dirs8", [116, 2, 3, NV], F8E4, kind="ExternalInput")
    scal_d = nc.dram_tensor("scal", [116, 2], F32, kind="ExternalInput")
    wt_d = nc.dram_tensor("wt", [NJ, NV], BF16, kind="ExternalInput")
    js2_d = nc.dram_tensor("js2", [11, 72], F32, kind="ExternalInput")
    ident_d = nc.dram_tensor("ident", [128, 128], F32, kind="ExternalInput")
    out_d = nc.dram_tensor("out", [B_LOC, 3, NV], BF16, kind="ExternalOutput")

    with tile.TileContext(nc) as tc:
        with (
            tc.tile_pool(name="const", bufs=1) as constp,
            tc.tile_pool(name="state", bufs=1) as statep,
            tc.tile_pool(name="scr", bufs=1) as scrp,
        ):
            # ---- const loads (pose/beta first: they gate the serial preamble) ----
            pose_sb = statep.tile([B_LOC, 72], F32)
            nc.sync.dma_start(pose_sb[:, :], pose_d.ap())
            coeff = statep.tile([B_LOC, KC], F32)
            nc.sync.dma_start(coeff[:, 1:11], beta_d.ap())
            nc.sync.dma_start(coeff[:, 12:22], beta_d.ap())
            ident = constp.tile([128, 128], F32)
            nc.sync.dma_start(ident[:, :], ident_d.ap())
            js2_sb = constp.tile([11, 72], F32)
            nc.sync.dma_start(js2_sb[:, :], js2_d.ap())
            scal_sb = constp.tile([116, 2], F32)
            nc.sync.dma_start(scal_sb[:, :], scal_d.ap())
            wt_sb = constp.tile([NJ, NV], BF16)
            nc.sync.dma_start(wt_sb[:, :], wt_d.ap())

            # ---- Rodrigues (fp32) ----
            V = nc.vector
            S = nc.scalar
            sq = scrp.tile([B_LOC, 72], F32, tag="sq")
            V.tensor_mul(sq[:, :], pose_sb[:, :], pose_sb[:, :])
            sq3 = sq[:, :].rearrange("p (j c) -> p c j", c=3)
            th2 = scrp.tile([B_LOC, NJ], F32, tag="th2")
            V.tensor_add(th2[:, :], sq3[:, 0, :], sq3[:, 1, :])
            V.tensor_add(th2[:, :], th2[:, :], sq3[:, 2, :])
            cbias = constp.tile([128, 3], F32)
            V.memset(cbias[:, 0:1], 1e-8)
            V.memset(cbias[:, 1:2], float(np.pi / 2))
            V.memset(cbias[:, 2:3], -1.0)
            theta = scrp.tile([B_LOC, NJ], F32, tag="theta")
            S.activation(theta[:, :], th2[:, :], mybir.ActivationFunctionType.Sqrt,
                         bias=cbias[0:B_LOC, 0:1])
            invt = scrp.tile([B_LOC, NJ], F32, tag="invt")
            V.reciprocal(invt[:, :], theta[:, :])
            sh = scrp.tile([B_LOC, NJ], F32, tag="sh")
            S.activation(sh[:, :], theta[:, :], mybir.ActivationFunctionType.Sin, scale=0.5)
            chh = scrp.tile([B_LOC, NJ], F32, tag="chh")
            S.activation(chh[:, :], theta[:, :], mybir.ActivationFunctionType.Sin,
                         scale=0.5, bias=cbias[0:B_LOC, 1:2])
            s_t = scrp.tile([B_LOC, NJ], F32, tag="s_t")
            V.scalar_tensor_tensor(s_t[:, :], sh[:, :], 2.0, chh[:, :], AluOpType.mult, AluOpType.mult)
            shsq = scrp.tile([B_LOC, NJ], F32, tag="shsq")
            V.tensor_mul(shsq[:, :], sh[:, :], sh[:, :])
            c_t = scrp.tile([B_LOC, NJ], F32, tag="c_t")
            S.activation(c_t[:, :], shsq[:, :], mybir.ActivationFunctionType.Copy,
                         bias=1.0, scale=-2.0)
            omc = scrp.tile([B_LOC, NJ], F32, tag="omc")
            S.mul(omc[:, :], shsq[:, :], 2.0)
            ax = scrp.tile([B_LOC, 72], F32, tag="ax")
            ax3 = ax[:, :].rearrange("p (j c) -> p c j", c=3)
            p3 = pose_sb[:, :].rearrange("p (j c) -> p c j", c=3)
            V.tensor_mul(ax3[:, :, :], p3[:, :, :],
                         invt[:, :].unsqueeze(1).broadcast_to([B_LOC, 3, NJ]))
            # products in one tile: [xx yy zz xy xz yz sx sy sz]
            t9 = scrp.tile([B_LOC, 9, NJ], F32, tag="t9")
            V.tensor_mul(t9[:, 0:3, :], ax3[:, :, :], ax3[:, :, :])
            V.tensor_mul(t9[:, 3:5, :],
                         ax3[:, 0:1, :].broadcast_to([B_LOC, 2, NJ]), ax3[:, 1:3, :])
            V.tensor_mul(t9[:, 5:6, :], ax3[:, 1:2, :], ax3[:, 2:3, :])
            V.tensor_mul(t9[:, 0:6, :], t9[:, 0:6, :],
                         omc[:, :].unsqueeze(1).broadcast_to([B_LOC, 6, NJ]))
            V.tensor_mul(t9[:, 6:9, :],
                         s_t[:, :].unsqueeze(1).broadcast_to([B_LOC, 3, NJ]),
                         ax3[:, :, :])
            prods = {n: t9[:, i, :] for i, n in enumerate(
                ["xx", "yy", "zz", "xy", "xz", "yz", "sx", "sy", "sz"])}
            r9 = statep.tile([B_LOC, NJ * 9], F32)
            r9e = r9[:, :].rearrange("p (j e) -> p e j", e=9)
            ENTRIES = [
                ("add", "c", "xx"), ("sub", "xy", "sz"), ("add", "xz", "sy"),
                ("add", "xy", "sz"), ("add", "c", "yy"), ("sub", "yz", "sx"),
                ("sub", "xz", "sy"), ("add", "yz", "sx"), ("add", "c", "zz"),
            ]
            for e, (op, a, b_) in enumerate(ENTRIES):
                ta = c_t[:, :] if a == "c" else prods[a]
                fn = V.tensor_add if op == "add" else V.tensor_sub
                fn(r9e[:, e, :], ta, prods[b_])

            # ---- coeff: [1|beta]hi [1|beta]lo lrotmin pad ----
            V.memset(coeff[:, 0:1], 1.0)
            V.memset(coeff[:, 11:12], 1.0)
            V.memset(coeff[:, 229:232], 0.0)
            S.copy(coeff[:, 22:229], r9[:, 9:216])
            lr9 = coeff[:, 22:229].rearrange("p (j e) -> p e j", e=9)
            for e in (0, 4, 8):
                S.add(lr9[:, e, :], lr9[:, e, :], cbias[0:B_LOC, 2:3])

            with tc.tile_pool(name="psA", bufs=2, space="PSUM") as psA:
                # full-coeff fp8 DoubleRow lhsT: expanded rows 0..115 from the
                # first transpose window, 116..231 from the second; per-row
                # power-of-2 scales delivered via scal_sb
                ptA = psA.tile([128, 128], F32, tag="tp")
                nc.tensor.transpose(ptA[:, :], coeff[:, 0:128], ident[:, :])
                betaT1 = statep.tile([11, B_LOC], F32)
                S.copy(betaT1[:, :], ptA[0:11, :])
                coeffT_b8 = statep.tile([116, 2, B_LOC], F8E4)
                S.mul(coeffT_b8[:, 0, :], ptA[0:116, :], scal_sb[:, 0:1])
                ptB = psA.tile([128, 128], F32, tag="tp")
                nc.tensor.transpose(ptB[0:116, :], coeff[:, 116:232], ident[:, :])
                S.mul(coeffT_b8[:, 1, :], ptB[0:116, :], scal_sb[:, 1:2])

                # ---- J = [1 | beta] @ JS2 (host-precomputed regressor) ----
                pj = psA.tile([B_LOC, 72], F32, tag="pj")
                nc.tensor.matmul(pj[:, :], betaT1[:, :], js2_sb[:, :],
                                 start=True, stop=True)
                j_sb = statep.tile([B_LOC, 72], F32)
                S.copy(j_sb[:, :], pj[:, :])

            # ---- J_rel ----
            jrel = statep.tile([B_LOC, 72], F32)
            jv = j_sb[:, :].rearrange("p (j c) -> p j c", c=3)
            jrv = jrel[:, :].rearrange("p (j c) -> p j c", c=3)
            S.copy(jrel[:, 0:3], j_sb[:, 0:3])
            V.tensor_sub(jrv[:, 1:4], jv[:, 1:4], jv[:, 0:1].broadcast_to([B_LOC, 3, 3]))
            V.tensor_sub(jrv[:, 4:12], jv[:, 4:12], jv[:, 1:9])
            V.tensor_sub(jrv[:, 12:15], jv[:, 12:15], jv[:, 9:10].broadcast_to([B_LOC, 3, 3]))
            V.tensor_sub(jrv[:, 15:18], jv[:, 15:18], jv[:, 12:15])
            V.tensor_sub(jrv[:, 18:24], jv[:, 18:24], jv[:, 16:22])

            # ---- local transforms Gl [128, 24*12] (3x4 row-major [R|t]) ----
            gl = statep.tile([B_LOC, NJ * 12], F32)
            gl4 = gl[:, :].rearrange("p (j m n) -> p j m n", m=3, n=4)
            r94 = r9[:, :].rearrange("p (j m n) -> p j m n", m=3, n=3)
            S.copy(gl4[:, :, :, 0:3], r94[:, :, :, :])
            S.copy(gl4[:, :, :, 3:4], jrv[:, :, :].unsqueeze(3))

            # ---- forward kinematics ----
            gw = statep.tile([B_LOC, NJ * 12], F32)
            gw4 = gw[:, :].rearrange("p (j m n) -> p j m n", m=3, n=4)
            S.copy(gw[:, 0:12], gl[:, 0:12])
            fk3 = scrp.tile([B_LOC, 3 * 3 * 12], F32, tag="fk3")
            for (c0, ncld, p0, bc) in FK_GROUPS:
                child = gw4[:, c0:c0 + ncld]
                loc = gl4[:, c0:c0 + ncld]
                par = gw4[:, p0:p0 + (1 if bc else ncld)]
                shp = [B_LOC, ncld, 3, 4]
                # per-k muls into a k-major scratch (APs are limited to 3
                # free dims), then tree adds
                f3 = fk3[:, 0:3 * ncld * 12].rearrange(
                    "p (k j m n) -> p k j m n", k=3, m=3, n=4)
                for k in range(3):
                    in0 = loc[:, :, k:k + 1, :].broadcast_to(shp)
                    pk = par[:, 0:1, :, k:k + 1] if bc else par[:, :, :, k:k + 1]
                    V.tensor_mul(f3[:, k], in0, pk.broadcast_to(shp))
                V.tensor_add(child[:, :, :, :], f3[:, 0], f3[:, 1])
                V.tensor_add(child[:, :, :, :], child[:, :, :, :], f3[:, 2])
                ptr = par[:, 0:1, :, 3:4] if bc else par[:, :, :, 3:4]
                V.tensor_add(child[:, :, :, 3:4], child[:, :, :, 3:4],
                             ptr.broadcast_to([B_LOC, ncld, 3, 1]))

            # ---- rest-pose correction: t_j -= R_j^w @ J_j (k-major fused) ----
            ct3 = scrp.tile([B_LOC, 3 * NJ * 3], F32, tag="ct3")
            c3v = ct3[:, :].rearrange("p (k j m) -> p k j m", k=3, m=3)
            cshp = [B_LOC, 3, NJ, 3]
            V.tensor_mul(c3v,
                         gw4[:, :, :, 0:3].rearrange("p j m k -> p k j m"),
                         jv[:, :, :].rearrange("p j k -> p k j").unsqueeze(3)
                         .broadcast_to(cshp))
            V.tensor_add(c3v[:, 0], c3v[:, 0], c3v[:, 1])
            V.tensor_add(c3v[:, 0], c3v[:, 0], c3v[:, 2])
            V.tensor_sub(gw4[:, :, :, 3:4], gw4[:, :, :, 3:4],
                         c3v[:, 0].unsqueeze(3))

            # ---- gat via 12 transposes: [24, 12*128] bf16 ----
            gat = statep.tile([NJ, 12 * B_LOC], BF16)
            gwe = gw[:, :].rearrange("p (j e) -> p e j", e=12)
            with tc.tile_pool(name="psT", bufs=3, space="PSUM") as psT:
                for g in range(6):
                    pgt = psT.tile([NJ, 2, B_LOC], F32, tag="gt")
                    for h in range(2):
                        nc.tensor.transpose(pgt[:, h, :], gwe[:, 2 * g + h, :],
                                            ident[:, :])
                    S.copy(
                        gat[:, 2 * g * B_LOC:(2 * g + 2) * B_LOC]
                        .rearrange("p (h v) -> p h v", v=B_LOC),
                        pgt[:, :, :])

            # ---- main vertex-chunk loop ----
            _main_loop(nc, tc, coeffT_b8, gat, wt_sb, dirs8_d, out_d)

    nc.compile()
    _CACHE[key] = nc
    return nc


def _main_loop(nc, tc, coeffT_b8, gat, wt_sb, dirs8_d, out_d):
    V = nc.vector
    S = nc.scalar
    ch = CH
    n_chunks = (NV + ch - 1) // ch

    with (
        tc.tile_pool(name="loop", bufs=4) as loopp,
        tc.tile_pool(name="pair", bufs=2) as pairp,
        tc.tile_pool(name="psVP", bufs=1, space="PSUM") as psVP,   # 3 banks
        tc.tile_pool(name="psTT", bufs=2, space="PSUM") as psTT,   # 2x2 banks
    ):
        state = {}

        def front_half(ci):
            """dirs DMA + vp matmuls + vp copy (ScalarE) for chunk ci.

            vp lands in slot ci%2 of a pair-wide tile so the combine can run
            double-FD instructions over two adjacent chunks."""
            v0 = ci * ch
            sz = min(ch, NV - v0)
            k = ci % 2
            if k == 0:
                state[("vp", ci // 2)] = pairp.tile([B_LOC, 3, 2, ch], BF16,
                                                    tag="vp", name="vp_pair")
            vp_pair = state[("vp", ci // 2)]
            db8 = loopp.tile([116, 2, 3, ch], F8E4, tag="db8")
            nc.sync.dma_start(db8[:, :, :, 0:sz], dirs8_d.ap()[:, :, :, v0:v0 + sz])
            pvp = psVP.tile([128, 3 * ch], F32, tag="pvp")
            for c in range(3):
                nc.tensor.matmul(pvp[:, c * ch:c * ch + sz], coeffT_b8[:, :, :],
                                 db8[:, :, c, 0:sz], start=True, stop=True,
                                 perf_mode=mybir.MatmulPerfMode.DoubleRow)
            if sz == ch:
                S.copy(vp_pair[:, :, k, :],
                       pvp[:, :].rearrange("p (c v) -> p c v", v=ch))
            else:
                for c in range(3):
                    S.copy(vp_pair[:, c, k, 0:sz], pvp[:, c * ch:c * ch + sz])

        def t_half(ci):
            """T matmuls + PSUM->SBUF copies for chunk ci into slot ci%2."""
            v0 = ci * ch
            sz = min(ch, NV - v0)
            k = ci % 2
            if k == 0:
                state[("t", ci // 2)] = pairp.tile([B_LOC, 12, 2, ch], BF16,
                                                   tag="t", name="t_pair")
            t_pair = state[("t", ci // 2)]
            for g in range(6):
                ptt = psTT.tile([B_LOC, 2 * ch], F32, tag="ptt")
                for h in range(2):
                    e = g * 2 + h
                    nc.tensor.matmul(ptt[:, h * ch:h * ch + sz],
                                     gat[:, e * B_LOC:(e + 1) * B_LOC],
                                     wt_sb[:, v0:v0 + sz], start=True, stop=True)
                if sz == ch:
                    S.copy(t_pair[:, g * 2:g * 2 + 2, k, :],
                           ptt[:, :].rearrange("p (h v) -> p h v", v=ch))
                else:
                    for h in range(2):
                        S.copy(t_pair[:, g * 2 + h, k, 0:sz],
                               ptt[:, h * ch:h * ch + sz])

        def combine(ci0, n_in_pair, szs):
            """double-FD combine over the pair (ci0, ci0+1); n_in_pair=1 for
            a lone tail chunk. szs: sizes of the pair's chunks."""
            v0 = ci0 * ch
            vp_pair = state.pop(("vp", ci0 // 2))
            t_pair = state.pop(("t", ci0 // 2))
            full = (n_in_pair == 2 and szs[1] == ch)
            w = 2 * ch if full else szs[0]  # combined free width when full
            pmul = pairp.tile([B_LOC, 3, 3, 2, ch], BF16, tag="pmul")
            q = pairp.tile([B_LOC, 3, 2, ch], BF16, tag="q")
            out_sb = pairp.tile([B_LOC, 3, 2, ch], BF16, tag="outsb")
            t3 = t_pair[:, :, :, :].rearrange("p (m n) k v -> p m n k v", n=4)[:, :, 3]

            if full:
                for c in range(3):
                    for m in range(3):
                        V.tensor_mul(pmul[:, c, m, :, :], t_pair[:, m * 4 + c, :, :],
                                     vp_pair[:, c, :, :])
                V.tensor_add(q[:, :, :, :], pmul[:, 0], pmul[:, 1])
                V.tensor_add(q[:, :, :, :], q[:, :, :, :], pmul[:, 2])
                V.tensor_add(out_sb[:, :, :, :], q[:, :, :, :], t3)
                nc.sync.dma_start(
                    out_d.ap()[:, :, v0:v0 + 2 * ch],
                    out_sb[:, :, :, :].rearrange("p c k v -> p c (k v)"))
            else:
                for kk in range(n_in_pair):
                    sz = szs[kk]
                    for c in range(3):
                        for m in range(3):
                            V.tensor_mul(pmul[:, c, m, kk, 0:sz],
                                         t_pair[:, m * 4 + c, kk, 0:sz],
                                         vp_pair[:, c, kk, 0:sz])
                    V.tensor_add(q[:, :, kk, 0:sz], pmul[:, 0, :, kk, 0:sz],
                                 pmul[:, 1, :, kk, 0:sz])
                    V.tensor_add(q[:, :, kk, 0:sz], q[:, :, kk, 0:sz],
                                 pmul[:, 2, :, kk, 0:sz])
                    V.tensor_add(out_sb[:, :, kk, 0:sz], q[:, :, kk, 0:sz],
                                 t3[:, :, kk, 0:sz])
                    nc.sync.dma_start(out_d.ap()[:, :, v0 + kk * ch:v0 + kk * ch + sz],
                                      out_sb[:, :, kk, 0:sz])

        # software pipeline, one chunk of lookahead on the vp side; combine
        # fires once per pair with double-FD DVE instructions.
        front_half(0)
        for ci in range(1, n_chunks):
            front_half(ci)
            t_half(ci - 1)
            if ci % 2 == 0:
                combine(ci - 2, 2, [ch, ch])
        t_half(n_chunks - 1)
        if n_chunks % 2 == 0:
            last0 = n_chunks - 2
            szs = [ch, min(ch, NV - (n_chunks - 1) * ch)]
            combine(last0, 2, szs)
        else:
            combine(n_chunks - 1, 1, [min(ch, NV - (n_chunks - 1) * ch), 0])


def _host_prep(inputs):
    shapedirs = np.asarray(inputs["shapedirs"], np.float32)    # [V,3,10]
    posedirs = np.asarray(inputs["posedirs"], np.float32)      # [V,3,207]
    v_template = np.asarray(inputs["v_template"], np.float32)  # [V,3]
    Jreg = np.asarray(inputs["J_regressor"], np.float32)       # [24,V]
    weights = np.asarray(inputs["weights"], np.float32)        # [V,24]

    def f8(x):
        return np.asarray(x, NP_F8E4)

    def dec(x):
        return f8(x).astype(np.float32)

    tmplr = v_template.T                      # [3, NV]
    sd = shapedirs.transpose(2, 1, 0)         # [10, 3, NV]
    pd = posedirs.transpose(2, 1, 0)          # [207, 3, NV]
    rows = np.zeros((232, 3, NV), np.float32)
    rows[0] = tmplr                           # hi rows store value at its scale
    rows[1:11] = 16 * sd
    rows[11] = 32 * (tmplr - dec(tmplr))      # residual rows
    rows[12:22] = 16 * (16 * sd - dec(16 * sd))
    rows[22:229] = 16 * pd
    dirs8 = f8(rows).reshape(2, 116, 3, NV).transpose(1, 0, 2, 3)
    scal = np.zeros(232, np.float32)
    scal[0] = 1.0
    scal[1:11] = 1.0 / 16
    scal[11] = 1.0 / 32
    scal[12:22] = 1.0 / 256
    scal[22:229] = 1.0 / 16
    scal2 = np.ascontiguousarray(scal.reshape(2, 116).T)
    # JS2: row 0 = Jreg @ template; rows 1..10 = Jreg @ shapedirs[:,:,s]
    js2 = np.empty((11, 72), np.float32)
    js2[0] = (Jreg @ v_template).reshape(72)
    js2[1:11] = np.einsum('jv,vcs->sjc', Jreg, shapedirs).reshape(10, 72)
    rep = {
        "dirs8": np.ascontiguousarray(dirs8),
        "scal": scal2,
        "wt": np.ascontiguousarray(weights.T.astype(NP_BF16)),
        "js2": js2,
        "ident": np.eye(128, dtype=np.float32),
    }
    return rep


def kernel(pose, beta, shapedirs, posedirs, v_template, J_regressor, weights):
    cfg = CFG
    nc = build_program(cfg)
    rep = _host_prep(dict(shapedirs=shapedirs, posedirs=posedirs, v_template=v_template,
                          J_regressor=J_regressor, weights=weights))
    pose = np.asarray(pose, np.float32)
    beta = np.asarray(beta, np.float32)
    in_maps = []
    for i in range(N_CORES):
        m = dict(rep)
        m["pose"] = np.ascontiguousarray(pose[i * B_LOC:(i + 1) * B_LOC])
        m["beta"] = np.ascontiguousarray(beta[i * B_LOC:(i + 1) * B_LOC])
        in_maps.append(m)
    res = run_bass_kernel_spmd(nc, in_maps, core_ids=list(range(N_CORES)),
                               trace=cfg.get("trace", False))
    kernel.last_results = res
    out = np.concatenate([np.asarray(res.results[i]["out"]).astype(np.float32)
                          for i in range(N_CORES)], axis=0)
    return np.ascontiguousarray(out.transpose(0, 2, 1))
